# revision 23
# baseline (speedup 1.0000x reference)
"""Trainium2 Bass kernel for nn_Network_38491496907327.

Computes, for X [65536, 512] f32 (with C1 = I, C2 = 1, C3 = 0 -- verified at
call time, exact-numpy fallback otherwise):

    quad = sum(X * X, axis=-1)                       # row-wise quadratic form
    y    = quad[:, None] + X
    out  = (y - mean_0(y)) / sqrt(var_0(y) + 1e-5)   # BatchNorm1d over axis 0

Distribution: data-parallel over rows, 8192 rows/core on 8 NeuronCores.
Batch statistics reduce to five sufficient statistics per shard
(colsum X, colsum q'X, colsum X^2; sum q', sum q'^2 with q' = quad - 512),
AllReduce-summed across cores; each core then normalizes its shard.

Per-core pipeline (64 row-tiles of [128, 512]):
  pass A: DMA X straight into the resident SBUF buffer (1MB super-chunks,
          contiguous 8KB per partition; the f32 bits are bitcast to f32r for
          the PE -- no round-copy). Square(X) -> x2 with an exact fp32
          row-sum accumulator (quad), alternating ScalarE/DVE so neither
          engine outruns the DMA stream. TensorE accumulates all shard
          stats into ONE PSUM tile [4,512]: rows 0-1 [ones|q']@X, row 2
          ones@x2, row 3 cols 0:2 the q' moments via a ones@[sum_t q',
          sum_t q'^2] matmul (fp32-exact path, no PE transpose, no DRAM
          partition hop).
  tail:  one ScalarE copy PSUM->SBUF, one 8KB DMA to DRAM, AllReduce,
         one 8KB DMA back (flat on partition 0).
  during the AllReduce wait, chunk-0's I@X matmuls are pre-issued into
  PSUM (they depend only on X).
  pass B: PE: psum = I@X + ones128 (x) c'  (c' = -colmean(X) - mean(q'));
          DVE: out = (psum + q') * invstd  -- one fused scalar_tensor_tensor
          per tile (q' rides the per-partition scalar slot, exact fp32);
          DMA out (1MB super-chunks).
"""

import sys

if "/opt/trn_rl_repo" not in sys.path:
    sys.path.insert(0, "/opt/trn_rl_repo")

import numpy as np

N = 65536
K = 512
NCORES = 8
ROWS = N // NCORES          # 8192 rows per core
P = 128                     # partitions
TILES = ROWS // P           # 64 row-tiles per core
SUP = 4                     # tiles per DMA super-chunk (1 MB)
NSUP = TILES // SUP         # 16 super-chunks
# chunk plan: (first_tile, n_tiles); last 1MB chunk splits into two 512KB
# chunks to halve the end-of-stream compute tail
CHUNKS = [(i * SUP, SUP) for i in range(NSUP - 1)] + \
    [((NSUP - 1) * SUP, 2), ((NSUP - 1) * SUP + 2, 2)]
USE_DSQRT = False  # no Dsqrt act table on TRN2
BN_EPS = 1e-5
QSHIFT = 512.0   # a-priori center of quad = ||x_row||^2 for x ~ N(0,1), K=512

_CACHE = {}


def _build(reps=1, serialize=True, ar=True):
    from concourse import bacc, tile, mybir

    F32 = mybir.dt.float32
    F32R = mybir.dt.float32r
    BF16 = mybir.dt.bfloat16
    ALU = mybir.AluOpType
    ACTF = mybir.ActivationFunctionType

    nc = bacc.Bacc("TRN2", target_bir_lowering=False, debug=False,
                   num_devices=NCORES)
    x_in = nc.dram_tensor("x", [ROWS, K], F32, kind="ExternalInput").ap()
    y_out = nc.dram_tensor("out", [ROWS, K], F32, kind="ExternalOutput").ap()
    ident_dram = nc.inline_tensor(np.eye(P, dtype=np.float32), name="ident")

    invN = 1.0 / float(N)

    with tile.TileContext(nc) as tc:
        with tc.tile_pool(name="sbuf", bufs=1) as pool, \
             tc.tile_pool(name="big", bufs=3) as bigpool, \
             tc.tile_pool(name="x2p", bufs=8) as x2pool, \
             tc.tile_pool(name="pps", bufs=1, space="PSUM") as pstat_pool, \
             tc.tile_pool(name="ppo", bufs=4, space="PSUM") as pout_pool, \
             tc.tile_pool(name="pab", bufs=1, space="PSUM") as pab_pool, \
             tc.tile_pool(name="dram", bufs=1, space="DRAM") as dram:
            # ---- constants ----
            ident_f = pool.tile([P, P], F32)
            nc.sync.dma_start(out=ident_f[:], in_=ident_dram.ap())
            ident_b = pool.tile([P, P], BF16)
            nc.scalar.copy(ident_b[:], ident_f[:])

            onescol = pool.tile([P, 1], F32)
            nc.vector.memset(onescol[:], 1.0)
            onescol_b = pool.tile([P, 1], BF16)
            nc.vector.memset(onescol_b[:], 1.0)
            onesrow_b = pool.tile([1, P], BF16)
            nc.vector.memset(onesrow_b[:], 1.0)
            epstp = pool.tile([P, 1], F32)
            nc.vector.memset(epstp[:], BN_EPS)
            epstp4 = pool.tile([P, 1], F32)
            nc.vector.memset(epstp4[:], BN_EPS / 4.0)

            def body():
                # ---- per-iteration state (bufs=1 pools: stable addresses) --
                xr_all = pool.tile([P, TILES * K], BF16, tag="xr_all")
                q_all = pool.tile([P, TILES], F32, tag="q_all")
                qq_cols = pool.tile([P, len(CHUNKS)], F32, tag="qq_cols")
                qscr = pool.tile([P, SUP], F32, tag="qscr")
                qro_all = pool.tile([P, TILES, 2], BF16, tag="qro_all")
                nc.vector.memset(qro_all[:], 1.0)  # pair col 0: ones
                ps_sxq = pstat_pool.tile([2, K], F32, tag="ps_sxq")
                ps_sxx = pstat_pool.tile([1, K], F32, tag="ps_sxx")
                ps_qm = pstat_pool.tile([1, 2], F32, tag="ps_qm")
                staging = pool.tile([2, K], F32, tag="staging")
                stag_sxx = pool.tile([1, K], F32, tag="stag_sxx")

                # ================= pass A =================
                # squares rotate over ScalarE/DVE/GpSimd so no single engine
                # falls behind the DMA stream; q' bookkeeping is per-chunk.
                # The final 4-tile chunk splits in two so the end-of-stream
                # compute tail is half as deep.
                rr = 0
                for ci, (t0, csz) in enumerate(CHUNKS):
                    tsl = slice(t0, t0 + csz)
                    dram_ap = x_in[t0 * P:(t0 + csz) * P, :] \
                        .rearrange("(p j) k -> p (j k)", p=P)
                    xsup = bigpool.tile([P, csz * K], F32, tag="big")
                    nc.sync.dma_start(out=xsup[:], in_=dram_ap)
                    x2s = []
                    tail = ci >= len(CHUNKS) - 2
                    for j in range(csz):
                        t = t0 + j
                        xt = xsup[:, j * K:(j + 1) * K]
                        xb = xr_all[:, t * K:(t + 1) * K]
                        x2 = x2pool.tile([P, K], BF16, tag="x2")
                        x2s.append(x2)
                        # per tile: one round-copy f32->bf16 (for the PE) and
                        # one fp32 Square with exact fp32 row-sum accumulate.
                        # Squares alternate ScalarE/DVE (the only engines
                        # with accumulate); copies ride mostly on GpSimd.
                        if j % 2 == 0:
                            nc.scalar.activation(x2[:], xt, ACTF.Square,
                                                 accum_out=q_all[:, t:t + 1])
                        else:
                            nc.vector.scalar_tensor_tensor(
                                out=x2[:], in0=xt, scalar=1.0, in1=xt,
                                op0=ALU.mult, op1=ALU.mult,
                                accum_out=q_all[:, t:t + 1])
                        if tail:
                            cp_eng = (nc.gpsimd, nc.vector)[j % 2]
                        else:
                            cp_eng = (nc.gpsimd, nc.gpsimd,
                                      nc.scalar, nc.gpsimd)[j % 4]
                        if cp_eng is nc.scalar:
                            nc.scalar.copy(xb, xt)
                        else:
                            cp_eng.tensor_copy(xb, xt)
                    # q' = quad - QSHIFT (exact; kills fp32 cancellation in
                    # Var(q) since quad ~ QSHIFT); chunk-granular so the
                    # post-stream tail only owes the last chunk
                    nc.vector.tensor_scalar_add(q_all[:, tsl], q_all[:, tsl],
                                                -QSHIFT)
                    nc.vector.tensor_copy(qro_all[:, tsl, 1:2],
                                          q_all[:, tsl].unsqueeze(2))
                    nc.vector.scalar_tensor_tensor(
                        out=qscr[:, 0:csz], in0=q_all[:, tsl], scalar=1.0,
                        in1=q_all[:, tsl], op0=ALU.mult, op1=ALU.mult,
                        accum_out=qq_cols[:, ci:ci + 1])
                    for j in range(csz):
                        t = t0 + j
                        first = (t == 0)
                        last = (t == TILES - 1)
                        nc.tensor.matmul(ps_sxq[:], qro_all[:, t, :],
                                         xr_all[:, t * K:(t + 1) * K],
                                         start=first, stop=last)
                        nc.tensor.matmul(ps_sxx[:], onescol_b[:], x2s[j][:],
                                         start=first, stop=last)

                # local q' moments, exact fp32: free-axis reduces -> [128,2],
                # ones-matmul partition reduce -> ps_qm [1,2]
                qsq = pool.tile([P, 2], F32, tag="qsq")
                nc.vector.tensor_reduce(qsq[:, 0:1], q_all[:],
                                        mybir.AxisListType.X, ALU.add)
                nc.vector.tensor_reduce(qsq[:, 1:2], qq_cols[:],
                                        mybir.AxisListType.X, ALU.add)
                nc.tensor.matmul(ps_qm[:], onescol[:], qsq[:],
                                 start=True, stop=True)

                # ---- stage stats -> AllReduce -> global stats ----
                # big rows and the tiny q-row ship on separate parallel DMAs
                STATS_W = 3 * K + 2
                nc.scalar.copy(staging[:], ps_sxq[:])
                nc.vector.tensor_copy(stag_sxx[:], ps_sxx[:])
                qm_sb = pool.tile([1, 2], F32, tag="qm_sb")
                nc.vector.tensor_copy(qm_sb[:], ps_qm[:])
                bounce_in = dram.tile([1, STATS_W], F32, tag="b_in")
                bounce_out = dram.tile([1, STATS_W], F32, tag="b_out")
                nc.sync.dma_start(out=bounce_in[:, 3 * K:], in_=qm_sb[:])
                nc.sync.dma_start(out=bounce_in[:, 2 * K:3 * K],
                                  in_=stag_sxx[:])
                nc.sync.dma_start(
                    out=bounce_in[:, 0:2 * K].rearrange(
                        "o (a b) -> (o a) b", a=2),
                    in_=staging[:])
                if ar:
                    nc.gpsimd.collective_compute(
                        "AllReduce", ALU.add,
                        replica_groups=[list(range(NCORES))],
                        ins=[bounce_in.opt()], outs=[bounce_out.opt()])
                else:  # timing probe: skip the collective (results 8x off)
                    nc.sync.dma_start(out=bounce_out.opt(),
                                      in_=bounce_in.opt())

                # chunk-0 I@X: depends only on X -- runs during the AR wait
                pouts0 = []
                for j in range(SUP):
                    pout = pout_pool.tile([P, K], F32, tag="po")
                    nc.tensor.matmul(pout[:], ident_b[:],
                                     xr_all[:, j * K:(j + 1) * K],
                                     start=True, stop=False)
                    pouts0.append(pout)

                gst = pool.tile([1, STATS_W], F32, tag="gst")
                nc.sync.dma_start(out=gst[:], in_=bounce_out.opt())
                Sx = gst[:, 0:K]
                Sqx = gst[:, K:2 * K]
                Sxx = gst[:, 2 * K:3 * K]
                Sq = gst[:, 3 * K:3 * K + 1]
                Sqq = gst[:, 3 * K + 1:3 * K + 2]

                # ---- derived vectors (partition 0) ----
                # var = Var(q') + 2*Cov(q',X) + Var(X)
                #     = (2*Sqx + Sxx)/N + s0 - invN^2 * Sx*(Sx + 2*Sq)
                # c' on DVE (unblocks the PE c'-matmuls early); var chain on
                # GpSimd; the [1,1] helpers go first so they are ready when
                # the row ops need them
                crow_b = pool.tile([1, K], BF16, tag="crow_b")
                nc.vector.tensor_scalar(out=crow_b[:], in0=Sx, scalar1=Sq,
                                        scalar2=-invN, op0=ALU.add,
                                        op1=ALU.mult)

                tsq = pool.tile([1, 1], F32, tag="tsq")  # 2*Sq
                nc.gpsimd.tensor_scalar_mul(tsq[:], Sq, 2.0)
                qbar = pool.tile([1, 1], F32, tag="qbar")
                nc.gpsimd.tensor_scalar_mul(qbar[:], Sq, invN)
                q2b = pool.tile([1, 1], F32, tag="q2b")
                nc.gpsimd.tensor_tensor(out=q2b[:], in0=qbar[:], in1=qbar[:],
                                        op=ALU.mult)
                s0 = pool.tile([1, 1], F32, tag="s0")   # Var(q')
                nc.gpsimd.tensor_scalar(out=s0[:], in0=Sqq, scalar1=invN,
                                        scalar2=q2b[:], op0=ALU.mult,
                                        op1=ALU.subtract)
                av = pool.tile([1, K], F32, tag="av")   # 2*Sqx + Sxx
                nc.vector.scalar_tensor_tensor(
                    out=av[:], in0=Sqx, scalar=2.0, in1=Sxx,
                    op0=ALU.mult, op1=ALU.add)
                tmp = pool.tile([1, K], F32, tag="tmp")  # Sx*(Sx + 2Sq)
                nc.vector.scalar_tensor_tensor(
                    out=tmp[:], in0=Sx, scalar=tsq[:], in1=Sx,
                    op0=ALU.add, op1=ALU.mult)
                w = pool.tile([1, K], F32, tag="w")
                nc.vector.tensor_scalar(out=w[:], in0=av[:], scalar1=invN,
                                        scalar2=s0[:], op0=ALU.mult,
                                        op1=ALU.add)
                varv = pool.tile([1, K], F32, tag="varv")
                nc.vector.scalar_tensor_tensor(
                    out=varv[:], in0=tmp[:], scalar=-invN * invN, in1=w[:],
                    op0=ALU.mult, op1=ALU.add)
                # hi/lo split keeps fp32 var precision through bf16 operands
                varv_hi = pool.tile([1, K], BF16, tag="varv_hi")
                nc.vector.tensor_copy(varv_hi[:], varv[:])
                varv_lo = pool.tile([1, K], BF16, tag="varv_lo")
                nc.vector.tensor_sub(varv_lo[:], varv[:], varv_hi[:])

                # broadcast var to [128, K] via K=1 outer products, then one
                # Sqrt + one reciprocal at full width gives 1/sd broadcast
                pab = pab_pool.tile([P, K], F32, tag="pab")
                nc.tensor.matmul(pab[:], onesrow_b[:], varv_hi[:],
                                 start=True, stop=False)
                nc.tensor.matmul(pab[:], onesrow_b[:], varv_lo[:],
                                 start=False, stop=True)
                abct = pool.tile([P, K], F32, tag="abct")
                if USE_DSQRT:
                    nc.scalar.activation(abct[:], pab[:], ACTF.Dsqrt,
                                         bias=epstp4[:], scale=0.25)
                else:
                    sdb = pool.tile([P, K], F32, tag="sdb")
                    nc.scalar.activation(sdb[:], pab[:], ACTF.Sqrt,
                                         bias=epstp[:])
                    nc.vector.reciprocal(abct[:], sdb[:])

                # ================= pass B =================
                # GpSimd cannot read PSUM, so the normalize splits as:
                # ScalarE Identity (scale*psum + per-partition q' bias) into
                # the output buffer, then DVE multiplies by 1/sd in place;
                # the first chunks DMA per tile so the output stream starts
                # as early as possible
                for ci, (t0, csz) in enumerate(CHUNKS):
                    osup = bigpool.tile([P, csz * K], F32, tag="big")
                    out_ap = y_out[t0 * P:(t0 + csz) * P, :] \
                        .rearrange("(p j) k -> p (j k)", p=P)
                    for j in range(csz):
                        t = t0 + j
                        if ci == 0:
                            pout = pouts0[j]
                        else:
                            pout = pout_pool.tile([P, K], F32, tag="po")
                            nc.tensor.matmul(
                                pout[:], ident_b[:],
                                xr_all[:, t * K:(t + 1) * K],
                                start=True, stop=False)
                        nc.tensor.matmul(pout[:], onesrow_b[:], crow_b[:],
                                         start=False, stop=True)
                        osl = osup[:, j * K:(j + 1) * K]
                        nc.scalar.activation(osl, pout[:], ACTF.Identity,
                                             bias=q_all[:, t:t + 1])
                        nc.vector.tensor_tensor(out=osl, in0=osl,
                                                in1=abct[:], op=ALU.mult)
                        if ci < 4:
                            # per-tile DMAs: first bytes hit HBM as soon as
                            # the first tile is normalized, and no 4-tile
                            # barrier throttles the young stream
                            nc.sync.dma_start(
                                out=out_ap[:, j * K:(j + 1) * K], in_=osl)
                    if ci >= 4:
                        nc.sync.dma_start(out=out_ap, in_=osup[:])

            for r in range(reps):
                if serialize and r > 0:
                    tc.strict_bb_all_engine_barrier()
                body()

    nc.compile()
    return nc


def _get_nc():
    if "nc" not in _CACHE:
        _CACHE["nc"] = _build()
    return _CACHE["nc"]


def _fallback(X, C1, C2, C3):
    X64 = X.astype(np.float64)
    quad = np.einsum("nk,kj,nj->n", X64, C1.astype(np.float64), X64)
    y = quad[:, None] + C2.astype(np.float64) * X64 + C3.astype(np.float64)
    mean = y.mean(axis=0)
    var = ((y - mean) ** 2).mean(axis=0)
    return ((y - mean) / np.sqrt(var + BN_EPS)).astype(np.float32)


def kernel(X, C1, C2, C3):
    X = np.ascontiguousarray(np.asarray(X, dtype=np.float32))
    C1 = np.asarray(C1, dtype=np.float32)
    C2 = np.asarray(C2, dtype=np.float32)
    C3 = np.asarray(C3, dtype=np.float32)
    fast = (
        X.shape == (N, K)
        and C1.shape == (K, K)
        and np.array_equal(C1, np.eye(K, dtype=np.float32))
        and C2.shape == (K,) and np.all(C2 == 1.0)
        and np.all(C3 == 0.0)
    )
    if not fast:
        return _fallback(X, C1, C2, C3)

    from concourse.bass_utils import run_bass_kernel_spmd

    nc = _get_nc()
    in_maps = [{"x": X[i * ROWS:(i + 1) * ROWS]} for i in range(NCORES)]
    last_err = None
    for _ in range(3):  # devices occasionally report transient
        try:                        # NRT_EXEC_UNIT_UNRECOVERABLE; retry clears it
            res = run_bass_kernel_spmd(nc, in_maps, core_ids=list(range(NCORES)))
            return np.concatenate(
                [res.results[i]["out"] for i in range(NCORES)], axis=0)
        except Exception as e:  # noqa: BLE001
            last_err = e
    import warnings
    warnings.warn(f"bass path failed ({last_err}); using numpy fallback")
    return _fallback(X, C1, C2, C3)


# revision 24
# speedup vs baseline: 2.6333x; 2.6333x over previous
"""Trainium2 Bass kernel for nn_Network_38491496907327.

Computes, for X [65536, 512] f32 (with C1 = I, C2 = 1, C3 = 0 -- verified at
call time, exact-numpy fallback otherwise):

    quad = sum(X * X, axis=-1)                       # row-wise quadratic form
    y    = quad[:, None] + X
    out  = (y - mean_0(y)) / sqrt(var_0(y) + 1e-5)   # BatchNorm1d over axis 0

Distribution: data-parallel over rows, 8192 rows/core on 8 NeuronCores.
Batch statistics reduce to five sufficient statistics per shard
(colsum X, colsum q'X, colsum X^2; sum q', sum q'^2 with q' = quad - 512),
AllReduce-summed across cores; each core then normalizes its shard.

Per-core pipeline (64 row-tiles of [128, 512]):
  pass A: DMA X straight into the resident SBUF buffer (1MB super-chunks,
          contiguous 8KB per partition; the f32 bits are bitcast to f32r for
          the PE -- no round-copy). Square(X) -> x2 with an exact fp32
          row-sum accumulator (quad), alternating ScalarE/DVE so neither
          engine outruns the DMA stream. TensorE accumulates all shard
          stats into ONE PSUM tile [4,512]: rows 0-1 [ones|q']@X, row 2
          ones@x2, row 3 cols 0:2 the q' moments via a ones@[sum_t q',
          sum_t q'^2] matmul (fp32-exact path, no PE transpose, no DRAM
          partition hop).
  tail:  one ScalarE copy PSUM->SBUF, one 8KB DMA to DRAM, AllReduce,
         one 8KB DMA back (flat on partition 0).
  during the AllReduce wait, chunk-0's I@X matmuls are pre-issued into
  PSUM (they depend only on X).
  pass B: PE: psum = I@X + ones128 (x) c'  (c' = -colmean(X) - mean(q'));
          DVE: out = (psum + q') * invstd  -- one fused scalar_tensor_tensor
          per tile (q' rides the per-partition scalar slot, exact fp32);
          DMA out (1MB super-chunks).
"""

import sys

if "/opt/trn_rl_repo" not in sys.path:
    sys.path.insert(0, "/opt/trn_rl_repo")

import numpy as np

N = 65536
K = 512
NCORES = 8
ROWS = N // NCORES          # 8192 rows per core
P = 128                     # partitions
TILES = ROWS // P           # 64 row-tiles per core
SUP = 4                     # tiles per DMA super-chunk (1 MB)
NSUP = TILES // SUP         # 16 super-chunks
# chunk plan: (first_tile, n_tiles); last 1MB chunk splits into two 512KB
# chunks to halve the end-of-stream compute tail
CHUNKS = [(i * SUP, SUP) for i in range(NSUP - 1)] + \
    [((NSUP - 1) * SUP, 2), ((NSUP - 1) * SUP + 2, 2)]
USE_DSQRT = False  # no Dsqrt act table on TRN2
USE_GP = True      # use GpSimd/Pool for copies + tiny scalar ops
BN_EPS = 1e-5
QSHIFT = 512.0   # a-priori center of quad = ||x_row||^2 for x ~ N(0,1), K=512

_CACHE = {}


def _build(reps=1, serialize=True, ar=True):
    from concourse import bacc, tile, mybir

    F32 = mybir.dt.float32
    F32R = mybir.dt.float32r
    BF16 = mybir.dt.bfloat16
    ALU = mybir.AluOpType
    ACTF = mybir.ActivationFunctionType

    nc = bacc.Bacc("TRN2", target_bir_lowering=False, debug=False,
                   num_devices=NCORES)
    x_in = nc.dram_tensor("x", [ROWS, K], F32, kind="ExternalInput").ap()
    y_out = nc.dram_tensor("out", [ROWS, K], F32, kind="ExternalOutput").ap()
    ident_dram = nc.inline_tensor(np.eye(P, dtype=np.float32), name="ident")

    invN = 1.0 / float(N)

    with tile.TileContext(nc) as tc:
        with tc.tile_pool(name="sbuf", bufs=1) as pool, \
             tc.tile_pool(name="big", bufs=3) as bigpool, \
             tc.tile_pool(name="x2p", bufs=8) as x2pool, \
             tc.tile_pool(name="pps", bufs=1, space="PSUM") as pstat_pool, \
             tc.tile_pool(name="ppo", bufs=4, space="PSUM") as pout_pool, \
             tc.tile_pool(name="pab", bufs=1, space="PSUM") as pab_pool, \
             tc.tile_pool(name="dram", bufs=1, space="DRAM") as dram:
            # ---- constants ----
            ident_f = pool.tile([P, P], F32)
            nc.sync.dma_start(out=ident_f[:], in_=ident_dram.ap())
            ident_b = pool.tile([P, P], BF16)
            nc.scalar.copy(ident_b[:], ident_f[:])

            onescol = pool.tile([P, 1], F32)
            nc.vector.memset(onescol[:], 1.0)
            onescol_b = pool.tile([P, 1], BF16)
            nc.vector.memset(onescol_b[:], 1.0)
            onesrow_b = pool.tile([1, P], BF16)
            nc.vector.memset(onesrow_b[:], 1.0)
            epstp = pool.tile([P, 1], F32)
            nc.vector.memset(epstp[:], BN_EPS)
            epstp4 = pool.tile([P, 1], F32)
            nc.vector.memset(epstp4[:], BN_EPS / 4.0)

            def body():
                # ---- per-iteration state (bufs=1 pools: stable addresses) --
                xr_all = pool.tile([P, TILES * K], BF16, tag="xr_all")
                q_all = pool.tile([P, TILES], F32, tag="q_all")
                qq_cols = pool.tile([P, len(CHUNKS)], F32, tag="qq_cols")
                qscr = pool.tile([P, SUP], F32, tag="qscr")
                qro_all = pool.tile([P, TILES, 2], BF16, tag="qro_all")
                nc.vector.memset(qro_all[:], 1.0)  # pair col 0: ones
                ps_sxq = pstat_pool.tile([2, K], F32, tag="ps_sxq")
                ps_sxx = pstat_pool.tile([1, K], F32, tag="ps_sxx")
                ps_qm = pstat_pool.tile([1, 2], F32, tag="ps_qm")
                staging = pool.tile([2, K], F32, tag="staging")
                stag_sxx = pool.tile([1, K], F32, tag="stag_sxx")

                # ================= pass A =================
                # squares rotate over ScalarE/DVE/GpSimd so no single engine
                # falls behind the DMA stream; q' bookkeeping is per-chunk.
                # The final 4-tile chunk splits in two so the end-of-stream
                # compute tail is half as deep.
                rr = 0
                for ci, (t0, csz) in enumerate(CHUNKS):
                    tsl = slice(t0, t0 + csz)
                    dram_ap = x_in[t0 * P:(t0 + csz) * P, :] \
                        .rearrange("(p j) k -> p (j k)", p=P)
                    xsup = bigpool.tile([P, csz * K], F32, tag="big")
                    nc.sync.dma_start(out=xsup[:], in_=dram_ap)
                    x2s = []
                    tail = ci >= len(CHUNKS) - 2
                    for j in range(csz):
                        t = t0 + j
                        xt = xsup[:, j * K:(j + 1) * K]
                        xb = xr_all[:, t * K:(t + 1) * K]
                        x2 = x2pool.tile([P, K], BF16, tag="x2")
                        x2s.append(x2)
                        # per tile: one round-copy f32->bf16 (for the PE) and
                        # one fp32 Square with exact fp32 row-sum accumulate.
                        # Squares alternate ScalarE/DVE (the only engines
                        # with accumulate); copies ride mostly on GpSimd.
                        if j % 2 == 0:
                            nc.scalar.activation(x2[:], xt, ACTF.Square,
                                                 accum_out=q_all[:, t:t + 1])
                        else:
                            nc.vector.scalar_tensor_tensor(
                                out=x2[:], in0=xt, scalar=1.0, in1=xt,
                                op0=ALU.mult, op1=ALU.mult,
                                accum_out=q_all[:, t:t + 1])
                        if not USE_GP:
                            cp_eng = (nc.scalar, nc.vector)[j % 2]
                        elif tail:
                            cp_eng = (nc.gpsimd, nc.vector)[j % 2]
                        else:
                            cp_eng = (nc.gpsimd, nc.gpsimd,
                                      nc.scalar, nc.gpsimd)[j % 4]
                        if cp_eng is nc.scalar:
                            nc.scalar.copy(xb, xt)
                        else:
                            cp_eng.tensor_copy(xb, xt)
                    # q' = quad - QSHIFT (exact; kills fp32 cancellation in
                    # Var(q) since quad ~ QSHIFT); chunk-granular so the
                    # post-stream tail only owes the last chunk
                    nc.vector.tensor_scalar_add(q_all[:, tsl], q_all[:, tsl],
                                                -QSHIFT)
                    nc.vector.tensor_copy(qro_all[:, tsl, 1:2],
                                          q_all[:, tsl].unsqueeze(2))
                    nc.vector.scalar_tensor_tensor(
                        out=qscr[:, 0:csz], in0=q_all[:, tsl], scalar=1.0,
                        in1=q_all[:, tsl], op0=ALU.mult, op1=ALU.mult,
                        accum_out=qq_cols[:, ci:ci + 1])
                    for j in range(csz):
                        t = t0 + j
                        first = (t == 0)
                        last = (t == TILES - 1)
                        nc.tensor.matmul(ps_sxq[:], qro_all[:, t, :],
                                         xr_all[:, t * K:(t + 1) * K],
                                         start=first, stop=last)
                        nc.tensor.matmul(ps_sxx[:], onescol_b[:], x2s[j][:],
                                         start=first, stop=last)

                # local q' moments, exact fp32: free-axis reduces -> [128,2],
                # ones-matmul partition reduce -> ps_qm [1,2]
                qsq = pool.tile([P, 2], F32, tag="qsq")
                nc.vector.tensor_reduce(qsq[:, 0:1], q_all[:],
                                        mybir.AxisListType.X, ALU.add)
                nc.vector.tensor_reduce(qsq[:, 1:2], qq_cols[:],
                                        mybir.AxisListType.X, ALU.add)
                nc.tensor.matmul(ps_qm[:], onescol[:], qsq[:],
                                 start=True, stop=True)

                # ---- stage stats -> AllReduce -> global stats ----
                # big rows and the tiny q-row ship on separate parallel DMAs
                STATS_W = 3 * K + 2
                nc.scalar.copy(staging[:], ps_sxq[:])
                nc.vector.tensor_copy(stag_sxx[:], ps_sxx[:])
                qm_sb = pool.tile([1, 2], F32, tag="qm_sb")
                nc.vector.tensor_copy(qm_sb[:], ps_qm[:])
                bounce_in = dram.tile([1, STATS_W], F32, tag="b_in")
                bounce_out = dram.tile([1, STATS_W], F32, tag="b_out")
                nc.sync.dma_start(out=bounce_in[:, 3 * K:], in_=qm_sb[:])
                nc.sync.dma_start(out=bounce_in[:, 2 * K:3 * K],
                                  in_=stag_sxx[:])
                nc.sync.dma_start(
                    out=bounce_in[:, 0:2 * K].rearrange(
                        "o (a b) -> (o a) b", a=2),
                    in_=staging[:])
                if ar:
                    nc.gpsimd.collective_compute(
                        "AllReduce", ALU.add,
                        replica_groups=[list(range(NCORES))],
                        ins=[bounce_in.opt()], outs=[bounce_out.opt()])
                else:  # timing probe: skip the collective (results 8x off)
                    nc.sync.dma_start(out=bounce_out.opt(),
                                      in_=bounce_in.opt())

                # chunk-0 I@X: depends only on X -- runs during the AR wait
                pouts0 = []
                for j in range(SUP):
                    pout = pout_pool.tile([P, K], F32, tag="po")
                    nc.tensor.matmul(pout[:], ident_b[:],
                                     xr_all[:, j * K:(j + 1) * K],
                                     start=True, stop=False)
                    pouts0.append(pout)

                gst = pool.tile([1, STATS_W], F32, tag="gst")
                nc.sync.dma_start(out=gst[:], in_=bounce_out.opt())
                Sx = gst[:, 0:K]
                Sqx = gst[:, K:2 * K]
                Sxx = gst[:, 2 * K:3 * K]
                Sq = gst[:, 3 * K:3 * K + 1]
                Sqq = gst[:, 3 * K + 1:3 * K + 2]

                # ---- derived vectors (partition 0) ----
                # var = Var(q') + 2*Cov(q',X) + Var(X)
                #     = (2*Sqx + Sxx)/N + s0 - invN^2 * Sx*(Sx + 2*Sq)
                # c' on DVE (unblocks the PE c'-matmuls early); var chain on
                # GpSimd; the [1,1] helpers go first so they are ready when
                # the row ops need them
                crow_b = pool.tile([1, K], BF16, tag="crow_b")
                nc.vector.tensor_scalar(out=crow_b[:], in0=Sx, scalar1=Sq,
                                        scalar2=-invN, op0=ALU.add,
                                        op1=ALU.mult)

                sc = nc.gpsimd if USE_GP else nc.vector
                tsq = pool.tile([1, 1], F32, tag="tsq")  # 2*Sq
                sc.tensor_scalar_mul(tsq[:], Sq, 2.0)
                qbar = pool.tile([1, 1], F32, tag="qbar")
                sc.tensor_scalar_mul(qbar[:], Sq, invN)
                q2b = pool.tile([1, 1], F32, tag="q2b")
                sc.tensor_tensor(out=q2b[:], in0=qbar[:], in1=qbar[:],
                                 op=ALU.mult)
                s0 = pool.tile([1, 1], F32, tag="s0")   # Var(q')
                sc.tensor_scalar(out=s0[:], in0=Sqq, scalar1=invN,
                                 scalar2=q2b[:], op0=ALU.mult,
                                 op1=ALU.subtract)
                av = pool.tile([1, K], F32, tag="av")   # 2*Sqx + Sxx
                nc.vector.scalar_tensor_tensor(
                    out=av[:], in0=Sqx, scalar=2.0, in1=Sxx,
                    op0=ALU.mult, op1=ALU.add)
                tmp = pool.tile([1, K], F32, tag="tmp")  # Sx*(Sx + 2Sq)
                nc.vector.scalar_tensor_tensor(
                    out=tmp[:], in0=Sx, scalar=tsq[:], in1=Sx,
                    op0=ALU.add, op1=ALU.mult)
                w = pool.tile([1, K], F32, tag="w")
                nc.vector.tensor_scalar(out=w[:], in0=av[:], scalar1=invN,
                                        scalar2=s0[:], op0=ALU.mult,
                                        op1=ALU.add)
                varv = pool.tile([1, K], F32, tag="varv")
                nc.vector.scalar_tensor_tensor(
                    out=varv[:], in0=tmp[:], scalar=-invN * invN, in1=w[:],
                    op0=ALU.mult, op1=ALU.add)
                # hi/lo split keeps fp32 var precision through bf16 operands
                varv_hi = pool.tile([1, K], BF16, tag="varv_hi")
                nc.vector.tensor_copy(varv_hi[:], varv[:])
                varv_lo = pool.tile([1, K], BF16, tag="varv_lo")
                nc.vector.tensor_sub(varv_lo[:], varv[:], varv_hi[:])

                # broadcast var to [128, K] via K=1 outer products, then one
                # Sqrt + one reciprocal at full width gives 1/sd broadcast
                pab = pab_pool.tile([P, K], F32, tag="pab")
                nc.tensor.matmul(pab[:], onesrow_b[:], varv_hi[:],
                                 start=True, stop=False)
                nc.tensor.matmul(pab[:], onesrow_b[:], varv_lo[:],
                                 start=False, stop=True)
                abct = pool.tile([P, K], F32, tag="abct")
                if USE_DSQRT:
                    nc.scalar.activation(abct[:], pab[:], ACTF.Dsqrt,
                                         bias=epstp4[:], scale=0.25)
                else:
                    sdb = pool.tile([P, K], F32, tag="sdb")
                    nc.scalar.activation(sdb[:], pab[:], ACTF.Sqrt,
                                         bias=epstp[:])
                    nc.vector.reciprocal(abct[:], sdb[:])

                # ================= pass B =================
                # GpSimd cannot read PSUM, so the normalize splits as:
                # ScalarE Identity (scale*psum + per-partition q' bias) into
                # the output buffer, then DVE multiplies by 1/sd in place;
                # the first chunks DMA per tile so the output stream starts
                # as early as possible
                for ci, (t0, csz) in enumerate(CHUNKS):
                    osup = bigpool.tile([P, csz * K], F32, tag="big")
                    out_ap = y_out[t0 * P:(t0 + csz) * P, :] \
                        .rearrange("(p j) k -> p (j k)", p=P)
                    for j in range(csz):
                        t = t0 + j
                        if ci == 0:
                            pout = pouts0[j]
                        else:
                            pout = pout_pool.tile([P, K], F32, tag="po")
                            nc.tensor.matmul(
                                pout[:], ident_b[:],
                                xr_all[:, t * K:(t + 1) * K],
                                start=True, stop=False)
                        nc.tensor.matmul(pout[:], onesrow_b[:], crow_b[:],
                                         start=False, stop=True)
                        osl = osup[:, j * K:(j + 1) * K]
                        nc.scalar.activation(osl, pout[:], ACTF.Identity,
                                             bias=q_all[:, t:t + 1])
                        nc.vector.tensor_tensor(out=osl, in0=osl,
                                                in1=abct[:], op=ALU.mult)
                        if ci < 4:
                            # per-tile DMAs: first bytes hit HBM as soon as
                            # the first tile is normalized, and no 4-tile
                            # barrier throttles the young stream
                            nc.sync.dma_start(
                                out=out_ap[:, j * K:(j + 1) * K], in_=osl)
                    if ci >= 4:
                        nc.sync.dma_start(out=out_ap, in_=osup[:])

            for r in range(reps):
                if serialize and r > 0:
                    tc.strict_bb_all_engine_barrier()
                body()

    nc.compile()
    return nc


def _get_nc():
    if "nc" not in _CACHE:
        _CACHE["nc"] = _build()
    return _CACHE["nc"]


def _fallback(X, C1, C2, C3):
    X64 = X.astype(np.float64)
    quad = np.einsum("nk,kj,nj->n", X64, C1.astype(np.float64), X64)
    y = quad[:, None] + C2.astype(np.float64) * X64 + C3.astype(np.float64)
    mean = y.mean(axis=0)
    var = ((y - mean) ** 2).mean(axis=0)
    return ((y - mean) / np.sqrt(var + BN_EPS)).astype(np.float32)


def kernel(X, C1, C2, C3):
    X = np.ascontiguousarray(np.asarray(X, dtype=np.float32))
    C1 = np.asarray(C1, dtype=np.float32)
    C2 = np.asarray(C2, dtype=np.float32)
    C3 = np.asarray(C3, dtype=np.float32)
    fast = (
        X.shape == (N, K)
        and C1.shape == (K, K)
        and np.array_equal(C1, np.eye(K, dtype=np.float32))
        and C2.shape == (K,) and np.all(C2 == 1.0)
        and np.all(C3 == 0.0)
    )
    if not fast:
        return _fallback(X, C1, C2, C3)

    from concourse.bass_utils import run_bass_kernel_spmd

    nc = _get_nc()
    in_maps = [{"x": X[i * ROWS:(i + 1) * ROWS]} for i in range(NCORES)]
    last_err = None
    for _ in range(3):  # devices occasionally report transient
        try:                        # NRT_EXEC_UNIT_UNRECOVERABLE; retry clears it
            res = run_bass_kernel_spmd(nc, in_maps, core_ids=list(range(NCORES)))
            return np.concatenate(
                [res.results[i]["out"] for i in range(NCORES)], axis=0)
        except Exception as e:  # noqa: BLE001
            last_err = e
    import warnings
    warnings.warn(f"bass path failed ({last_err}); using numpy fallback")
    return _fallback(X, C1, C2, C3)
